# revision 1
# baseline (speedup 1.0000x reference)
"""Trainium2 Bass kernel for 16-head MHA (B=4, S=2048, E=1024, fp32).

Sharding: 8 cores = (batch b, head-half hh) grid. Core c handles batch
c // 2 and heads [hh*8, hh*8+8) (d-slice of 512 channels). Each core
computes a partial y_c = attn_out_slice @ Wo_slice.T of the full (S, E)
output; the host sums core pairs and adds bo.

Device kernel layout choices (fp32 storage, float32r matmul views):
  - xT (E, S) is staged host-side so projections contract E on partitions.
  - KT (d on partitions, S free) / V natural (S, 512) projected upfront;
    QT projected per 512-wide q-chunk inside the main loop (SBUF pressure).
  - scoresT (k on partitions, q free): per (qc, j, k): two row-tiled K=64
    matmuls (head pair) into a 2-bank psum group; one Exp over the
    combined (128, 1024) group with scale=1/8 (no max subtraction -- the
    logits for this problem are bounded ~|2|).
  - PV: col-tiled M=64 pairs accumulate over k into one bank (head0 ->
    partitions 0:64, head1 -> 64:128). Softmax denominator: partial sums
    over k on DVE, then a ones-stationary matmul reduces partitions and
    replicates to (128, 512); reciprocal_approx_fast + multiply + bv add.
    (V bias folds out of PV and is added post-normalization.)
  - O-projection per s-chunk contracts d_loc through out_cT tiles.
"""
import numpy as np

import concourse.bass as bass
import concourse.mybir as mybir
import concourse.tile as tile
from concourse import bacc
from concourse.bass_utils import run_bass_kernel_spmd

B, S, E = 4, 2048, 1024
DLOC = 512          # head-dim channels per core (8 heads)
NJ = DLOC // 128    # 4 j-chunks (head pairs)
NE = E // 128       # 8 e-chunks
NSC = S // 128      # 16 s-chunks
NQC = S // 512      # 4 q-chunks
NKC = S // 128      # 16 k-chunks
F32 = mybir.dt.float32
F32R = mybir.dt.float32r
EXP = mybir.ActivationFunctionType.Exp

_CACHED = {}


def _build(loop_k=None):
    nc = bacc.Bacc()
    xT = nc.declare_dram_parameter("xT", [E, S], F32R, isOutput=False)
    wqT = nc.declare_dram_parameter("wqT", [E, DLOC], F32R, isOutput=False)
    wkT = nc.declare_dram_parameter("wkT", [E, DLOC], F32R, isOutput=False)
    wvT = nc.declare_dram_parameter("wvT", [E, DLOC], F32R, isOutput=False)
    woT = nc.declare_dram_parameter("woT", [DLOC, E], F32R, isOutput=False)
    bq = nc.declare_dram_parameter("bq", [DLOC, 1], F32, isOutput=False)
    bk = nc.declare_dram_parameter("bk", [DLOC, 1], F32, isOutput=False)
    bv = nc.declare_dram_parameter("bv", [DLOC, 1], F32, isOutput=False)
    ones = nc.declare_dram_parameter("ones", [128, 64], F32R, isOutput=False)
    y = nc.declare_dram_parameter("y", [S, E], F32, isOutput=True)

    with tile.TileContext(nc) as tc:
        with (
            tc.tile_pool(name="big", bufs=1) as big,
            tc.tile_pool(name="wpool", bufs=2) as wpool,
            tc.tile_pool(name="cons", bufs=1) as cons,
            tc.tile_pool(name="qpool", bufs=1) as qpool,
            tc.tile_pool(name="opool", bufs=2) as opool,
            tc.tile_pool(name="ppool", bufs=2) as ppool,
            tc.tile_pool(name="dpool", bufs=1) as dpool,
            tc.tile_pool(name="ypool", bufs=1) as ypool,
            tc.tile_pool(name="ps_proj", bufs=2, space="PSUM") as ps_proj,
            tc.tile_pool(name="ps_sc", bufs=2, space="PSUM") as ps_sc,
            tc.tile_pool(name="ps_pv", bufs=2, space="PSUM") as ps_pv,
        ):
            # ---- constants ----
            ones64 = cons.tile([128, 64], F32R)
            nc.sync.dma_start(out=ones64, in_=ones[:, :])
            bq_t = cons.tile([128, NJ], F32)
            bk_t = cons.tile([128, NJ], F32)
            bv_t = cons.tile([128, NJ], F32)
            bvh1_t = cons.tile([64, NJ], F32)
            for j in range(NJ):
                nc.sync.dma_start(out=bq_t[:, j:j+1], in_=bq[j*128:(j+1)*128, :])
                nc.sync.dma_start(out=bk_t[:, j:j+1], in_=bk[j*128:(j+1)*128, :])
                nc.sync.dma_start(out=bv_t[:, j:j+1], in_=bv[j*128:(j+1)*128, :])
                nc.sync.dma_start(out=bvh1_t[:, j:j+1],
                                  in_=bv[j*128+64:(j+1)*128, :])
            
            # ---- optional on-device repeat loop (timing only) ----
            import contextlib
            loop_cm = tc.For_i(0, loop_k) if loop_k else contextlib.nullcontext()
            with loop_cm:
                _body(nc, tc, locals())

    nc.compile()
    return nc


def _body(nc, tc, env):
    xT, wqT, wkT, wvT, woT = env["xT"], env["wqT"], env["wkT"], env["wvT"], env["woT"]
    y = env["y"]
    big, wpool, qpool, opool = env["big"], env["wpool"], env["qpool"], env["opool"]
    ppool, dpool, ypool = env["ppool"], env["dpool"], env["ypool"]
    ps_proj, ps_sc, ps_pv = env["ps_proj"], env["ps_sc"], env["ps_pv"]
    ones64, bq_t, bk_t, bv_t, bvh1_t = (env["ones64"], env["bq_t"], env["bk_t"],
                                        env["bv_t"], env["bvh1_t"])

    # ---- weights first (small), then xT chunks ----
    wv_t = wpool.tile([128, NE, DLOC], F32R, tag="w")
    for e in range(NE):
        nc.sync.dma_start(out=wv_t[:, e, :], in_=wvT[e*128:(e+1)*128, :])
    wk_t = wpool.tile([128, NE, DLOC], F32R, tag="w")
    for e in range(NE):
        nc.sync.dma_start(out=wk_t[:, e, :], in_=wkT[e*128:(e+1)*128, :])
    xt_list = []
    for e in range(NE):
        xe = big.tile([128, S], F32R, tag=f"xt{e}")
        nc.sync.dma_start(out=xe, in_=xT[e*128:(e+1)*128, :])
        xt_list.append(xe)

    # ---- V projection (natural layout; bias folded out) ----
    # vt[:, sc, j, h, 0:64] = V columns; vt[:, sc, j, h, 64] = 1.0 so the
    # PV matmul's 65th output row accumulates the softmax denominator.
    vt = big.tile([128, NSC, NJ, 2, 65], F32R)
    nc.vector.tensor_copy(
        vt[:, :, :, :, 64:65],
        ones64[:, 0:1].broadcast_to((128, NSC, NJ, 2, 1)))
    def v_proj_group(sc):
        cell = {}
        def get_pv():
            if "pv" not in cell:
                pv_lazy = ps_proj.tile([128, 512], F32, tag="proj")
                cell["pv"] = pv_lazy
            return cell["pv"]
        mms = [lambda e=e: nc.tensor.matmul(
                   get_pv(), xt_list[e][:, sc*128:(sc+1)*128], wv_t[:, e, :],
                   start=(e == 0), stop=(e == NE - 1)) for e in range(NE)]
        def evac():
            nc.vector.tensor_copy(
                vt[:, sc, :, :, 0:64],
                get_pv().rearrange("p (j h c) -> p j h c", j=NJ, h=2))
        return mms, evac

    for sc in range(NSC):
        mms, evac = v_proj_group(sc)
        for m in mms:
            m()
        evac()

    wq_t = wpool.tile([128, NE, DLOC], F32R, tag="w")
    for e in range(NE):
        nc.sync.dma_start(out=wq_t[:, e, :], in_=wqT[e*128:(e+1)*128, :])
    wo_t = wpool.tile([128, NJ, E], F32R, tag="w")
    for j in range(NJ):
        nc.sync.dma_start(out=wo_t[:, j, :], in_=woT[j*128:(j+1)*128, :])

    oct_ = big.tile([128, NJ, S], F32R)

    # ---- main loop: j (head pairs) outer, q-chunks inner ----
    # Projections are software-pipelined into the attention k-loop ("fill"
    # slots) so the statically-scheduled PE stream never starves ACT.
    def k_proj_group(j, qc, w_t, b_t, dest_fn):
        cell = {}
        def get_pk():
            if "pk" not in cell:
                pk_lazy = ps_proj.tile([128, 512], F32, tag="proj")
                cell["pk"] = pk_lazy
            return cell["pk"]
        mms = [lambda e=e: nc.tensor.matmul(
                   get_pk(), w_t[:, e, j*128:(j+1)*128],
                   xt_list[e][:, qc*512:(qc+1)*512],
                   start=(e == 0), stop=(e == NE - 1)) for e in range(NE)]
        def evac():
            nc.vector.tensor_scalar_add(dest_fn(), get_pk(), b_t[:, j:j+1])
        return mms, evac

    def o_proj_sc(sc):
        cell = {}
        def get_ysb():
            if "ysb" not in cell:
                ysb_lazy = ypool.tile([128, E], F32, tag="y", bufs=2)
                cell["ysb"] = ysb_lazy
            return cell["ysb"]
        def get_py(eh):
            key = f"py{eh}"
            if key not in cell:
                py_lazy = ps_proj.tile([128, 512], F32, tag="proj")
                cell[key] = py_lazy
            return cell[key]
        steps = []
        for eh in range(2):
            for jj in range(NJ):
                steps.append(lambda jj=jj, eh=eh: nc.tensor.matmul(
                    get_py(eh), oct_[:, jj, sc*128:(sc+1)*128],
                    wo_t[:, jj, eh*512:(eh+1)*512],
                    start=(jj == 0), stop=(jj == NJ - 1)))
            steps.append(lambda eh=eh: nc.vector.tensor_copy(
                get_ysb()[:, eh*512:(eh+1)*512], get_py(eh)))
        steps.append(lambda: nc.sync.dma_start(
            out=y[sc*128:(sc+1)*128, :], in_=get_ysb()))
        return steps

    # K-projection for j=0 and Q-projection for (0, 0) run up front.
    kt_next = qpool.tile([128, S], F32R, tag="kt", bufs=2)
    for qc in range(NQC):
        mms, evac = k_proj_group(
            0, qc, wk_t, bk_t,
            (lambda qc=qc, t=kt_next: t[:, qc*512:(qc+1)*512]))
        for m in mms:
            m()
        evac()
    qt_next = qpool.tile([128, 512], F32R, tag="qt", bufs=2)
    mms, evac = q_proj_group = k_proj_group(
        0, 0, wq_t, bq_t, (lambda t=qt_next: t[:, :]))
    for m in mms:
        m()
    evac()

    for j in range(NJ):
        kt = kt_next
        if j < NJ - 1:
            kt_next = qpool.tile([128, S], F32R, tag="kt", bufs=2)
        for qc in range(NQC):
            qt = qt_next
            # fill work emitted one step per k iteration
            fills = []
            if qc < NQC - 1:
                qt_next = qpool.tile([128, 512], F32R, tag="qt", bufs=2)
                mms, evac = k_proj_group(
                    j, qc + 1, wq_t, bq_t, (lambda t=qt_next: t[:, :]))
                fills.extend(mms); fills.append(evac)
            elif j < NJ - 1:
                qt_next = qpool.tile([128, 512], F32R, tag="qt", bufs=2)
                mms, evac = k_proj_group(
                    j + 1, 0, wq_t, bq_t, (lambda t=qt_next: t[:, :]))
                fills.extend(mms); fills.append(evac)
            if j < NJ - 1:
                mms, evac = k_proj_group(
                    j + 1, qc, wk_t, bk_t,
                    (lambda qc=qc, t=kt_next: t[:, qc*512:(qc+1)*512]))
                fills.extend(mms); fills.append(evac)
            if j == NJ - 1 and qc > 0:
                for scl in range(4):
                    fills.extend(o_proj_sc((qc - 1) * 4 + scl))

            pvh0 = ps_pv.tile([65, 512], F32, tag="pv")
            pvh1 = ps_pv.tile([65, 512], F32, tag="pv")
            nf = len(fills)
            for k in range(NKC):
                sgrp = ps_sc.tile([128, 2, 512], F32, tag="sc")
                nc.tensor.matmul(
                    sgrp[:, 0, :], kt[0:64, k*128:(k+1)*128],
                    qt[0:64, :], start=True, stop=True)
                nc.tensor.matmul(
                    sgrp[:, 1, :], kt[64:128, k*128:(k+1)*128],
                    qt[64:128, :], start=True, stop=True)
                pgrp = ppool.tile([128, 2, 512], F32R, tag="p")
                nc.scalar.activation(pgrp[:, :, :], sgrp[:, :, :],
                                     EXP, scale=0.125)
                nc.tensor.matmul(
                    pvh0, vt[:, k, j, 0, :],
                    pgrp[:, 0, :], start=(k == 0), stop=(k == NKC - 1))
                nc.tensor.matmul(
                    pvh1, vt[:, k, j, 1, :],
                    pgrp[:, 1, :], start=(k == 0), stop=(k == NKC - 1))
                # drain fill work: ceil-spread across the 16 k slots
                lo = (nf * k) // NKC
                hi = (nf * (k + 1)) // NKC
                for f in fills[lo:hi]:
                    f()
            den0 = dpool.tile([1, 512], F32R, tag="den0")
            nc.vector.tensor_copy(den0, pvh0[64:65, :])
            den1 = dpool.tile([1, 512], F32R, tag="den1")
            nc.vector.tensor_copy(den1, pvh1[64:65, :])
            drep0 = ps_proj.tile([64, 512], F32, tag="proj")
            nc.tensor.matmul(drep0, ones64[0:1, :], den0,
                             start=True, stop=True)
            drep1 = ps_proj.tile([64, 512], F32, tag="proj")
            nc.tensor.matmul(drep1, ones64[0:1, :], den1,
                             start=True, stop=True)
            recip0 = dpool.tile([64, 512], F32, tag="recip")
            nc.vector.reciprocal_approx_fast(out=recip0, in_=drep0)
            recip1 = dpool.tile([64, 512], F32, tag="recip1")
            nc.vector.reciprocal_approx_fast(out=recip1, in_=drep1)
            nc.vector.tensor_mul(
                oct_[0:64, j, qc*512:(qc+1)*512], pvh0[0:64, :], recip0)
            nc.vector.tensor_scalar_add(
                oct_[0:64, j, qc*512:(qc+1)*512],
                oct_[0:64, j, qc*512:(qc+1)*512], bv_t[0:64, j:j+1])
            tmp1 = dpool.tile([64, 512], F32R, tag="tmp1")
            nc.vector.tensor_mul(tmp1, pvh1[0:64, :], recip1)
            nc.vector.tensor_scalar_add(tmp1, tmp1, bvh1_t[0:64, j:j+1])
            nc.sync.dma_start(out=oct_[64:128, j, qc*512:(qc+1)*512], in_=tmp1)

    # last q-chunk's output projection (tail)
    for scl in range(4):
        for f in o_proj_sc(12 + scl):
            f()


def _get_nc():
    if "nc" not in _CACHED:
        _CACHED["nc"] = _build()
    return _CACHED["nc"]


def kernel(x, Wq, bq, Wk, bk, Wv, bv, Wo, bo):
    x = np.asarray(x, dtype=np.float32)
    in_maps = []
    for c in range(8):
        b, hh = c // 2, c % 2
        hsel = slice(hh * DLOC, (hh + 1) * DLOC)
        in_maps.append({
            "xT": np.ascontiguousarray(x[b].T),
            "wqT": np.ascontiguousarray(np.asarray(Wq, dtype=np.float32)[hsel, :].T),
            "wkT": np.ascontiguousarray(np.asarray(Wk, dtype=np.float32)[hsel, :].T),
            "wvT": np.ascontiguousarray(np.asarray(Wv, dtype=np.float32)[hsel, :].T),
            "woT": np.ascontiguousarray(np.asarray(Wo, dtype=np.float32)[:, hsel].T),
            "bq": np.asarray(bq, dtype=np.float32)[hsel].reshape(DLOC, 1),
            "bk": np.asarray(bk, dtype=np.float32)[hsel].reshape(DLOC, 1),
            "bv": np.asarray(bv, dtype=np.float32)[hsel].reshape(DLOC, 1),
            "ones": np.ones((128, 64), dtype=np.float32),
        })
    nc = _get_nc()
    res = run_bass_kernel_spmd(nc, in_maps, list(range(8))).results
    out = np.empty((B, S, E), dtype=np.float32)
    bo = np.asarray(bo, dtype=np.float32)
    for b in range(B):
        out[b] = res[2 * b]["y"] + res[2 * b + 1]["y"] + bo
    return out

